# revision 49
# baseline (speedup 1.0000x reference)
"""Trainium2 Bass kernel for nn_DiscreteDiT (DiT backbone + pairwise MLP + Sinkhorn).

Sharding: 8 cores = 4 batches x 2 row-halves. Each core runs the full DiT
backbone for its batch (replicated within the pair), computes pairwise-MLP
edge logits for its half of the rows (row selection is data-driven via a
selection matrix S so the program is identical on every core), exchanges
halves with its pair core via a 2-rank AllGather, and runs the full Sinkhorn
normalisation redundantly.

kernel(**inputs) takes the FULL unsharded inputs and returns (la, x0), both
[B, N, N] float32, matching reference.reference().
"""

import math
import os
import sys

import numpy as np

if "/opt/trn_rl_repo" not in sys.path:
    sys.path.insert(0, "/opt/trn_rl_repo")

import concourse.bass as bass
import concourse.mybir as mybir
import concourse.tile as tile
from concourse import bacc
from concourse.bass import AP
from concourse.bass_utils import run_bass_kernel_spmd

# Restrict ACT table-set choice so Exp and Ln resolve to the combined
# natural_log_exp set (otherwise the placement pass ping-pongs between
# exp_and_others and natural_log on every Ln/Exp pair: ~100 extra 2.7us
# table loads).  Indices/order of act_info.json are preserved; we only
# empty the function sets we never want chosen.
_ACT_SETS_KEEP = {"natural_log_exp_and_others", "gelu_apprx_tanh_and_others"}
_orig_gat = bacc.get_activation_tables


def _gat_restricted(arch):
    tabs = _orig_gat(arch)
    return {k: (v if k in _ACT_SETS_KEEP else set()) for k, v in tabs.items()}


bacc.get_activation_tables = _gat_restricted

F32 = mybir.dt.float32
F32R = mybir.dt.float32r
BF16 = mybir.dt.bfloat16
AFT = mybir.ActivationFunctionType
ALU = mybir.AluOpType
AX = mybir.AxisListType

# Model dims (hardcoded per spec)
B = 4
N = 192          # tokens / nodes
IN_DIM = 1728
D = 256
HEADS = 4
HD = 64
LAYERS = 4
MLP_HID = 256    # pairwise-MLP hidden
SINK_ITERS = 20
TEMP = 0.05
NT = 96          # tokens per partition-chunk (N = 2*NT)
MY_ROWS = 96     # pairwise rows per core
IB = 2           # rows per pairwise i-block
NBLK = MY_ROWS // IB
PWN = IB * N     # pairwise matmul free dim = 384
IN_CHUNKS = [(kc * 128, min(128, IN_DIM - kc * 128)) for kc in range((IN_DIM + 127) // 128)]

USE_F32R = True
BF16_BB = True   # bf16 for qk/scores/v/m1 backbone matmuls (N=192 paths)
DIAG_SMALL = -88.0  # working-diagonal value (exp underflows to ~0, no overflow risk)


def _f(ap):
    """View an fp32 AP as float32r for full-rate matmul (moving dim >= 256)."""
    return ap.bitcast(F32R) if USE_F32R else ap


def _r(ap):
    """fp32r view for a producer's output AP (HW rounds on write)."""
    return ap.bitcast(F32R) if USE_F32R else ap


# --------------------------------------------------------------------------
# Device program
# --------------------------------------------------------------------------

def build_program(n_cores=8, enable_asserts=False, skip_collective=False):
    nc = bacc.Bacc(
        "TRN2",
        target_bir_lowering=False,
        debug=False,
        enable_asserts=enable_asserts,
        num_devices=n_cores,
    )

    def din(name, shape, dt=F32):
        return nc.dram_tensor(name, shape, dt, kind="ExternalInput").ap()

    inp = dict(
        xT=din("xT", [IN_DIM, N]),
        in_w=din("in_w", [IN_DIM, D]),
        in_b=din("in_b", [1, D]),
        W0=din("W0", [D, D]),
        W12=din("W12", [2 * D, D]),
        p0_b=din("p0_b", [128, 2]),
        p1_w=din("p1_w", [D, MLP_HID]),
        p1_b=din("p1_b", [128, 2]),
        po_w=din("po_w", [128, 2]),
        S=din("S", [N, MY_ROWS]),
        ident=din("ident", [128, 128]),
        M0p=din("M0p", [NT, 2 * N]),
        M1p=din("M1p", [NT, 2 * N]),
        M1fix=din("M1fix", [NT, 2 * N]),
    )
    lwl = []
    for l in range(LAYERS):
        lwl.append(dict(
            qk_bf=din(f"L{l}_qk_bf", [D, 2 * D], BF16),
            v_bf=din(f"L{l}_v_bf", [D, D], BF16),
            m1_bf=din(f"L{l}_m1_bf", [D, 4 * D], BF16),
            qkv_b=din(f"L{l}_qkv_b", [128, 6]),      # chunk-major columns
            o_w=din(f"L{l}_o_w", [D, D]),
            o_b=din(f"L{l}_o_b", [1, D]),
            m1_b=din(f"L{l}_m1_b", [128, 8]),
            m2_w=din(f"L{l}_m2_w", [4 * D, D]),
            m2_b=din(f"L{l}_m2_b", [1, D]),
            vb=din(f"L{l}_vb", [1, D]),
        ))

    la_d = nc.dram_tensor("la_out", [N, N], F32, kind="ExternalOutput").ap()
    x0_d = nc.dram_tensor("x0_out", [N, N], F32, kind="ExternalOutput").ap()
    xch_in = [nc.dram_tensor("xch_in_a", [MY_ROWS // 2, N], F32).ap(),
              nc.dram_tensor("xch_in_b", [MY_ROWS // 2, N], F32).ap()]
    xch_out = [nc.dram_tensor("xch_out_a", [MY_ROWS, N], F32).ap(),
               nc.dram_tensor("xch_out_b", [MY_ROWS, N], F32).ap()]

    from contextlib import ExitStack
    with tile.TileContext(nc) as tc:
        with ExitStack() as ctx:
            _body(ctx, tc, nc, inp, lwl, la_d, x0_d, xch_in, xch_out,
                  skip_collective=skip_collective)
    nc.compile()
    return nc


def _body(ctx, tc, nc, inp, lwl, la_d, x0_d, xch_in, xch_out,
          skip_collective=False):
    dma = nc.sync.dma_start
    TT = nc.vector.tensor_tensor
    TS = nc.vector.tensor_scalar
    act = nc.scalar.activation

    const = ctx.enter_context(tc.tile_pool(name="const", bufs=1))
    wpool = ctx.enter_context(tc.tile_pool(name="weights", bufs=1))
    lqkv = ctx.enter_context(tc.tile_pool(name="lqkv", bufs=2))
    lo = ctx.enter_context(tc.tile_pool(name="lo", bufs=2))
    lm1 = ctx.enter_context(tc.tile_pool(name="lm1", bufs=2))
    lm2 = ctx.enter_context(tc.tile_pool(name="lm2", bufs=2))
    lsmall = ctx.enter_context(tc.tile_pool(name="lsmall", bufs=2))
    hpool = ctx.enter_context(tc.tile_pool(name="hstate", bufs=1))
    work = ctx.enter_context(tc.tile_pool(name="work", bufs=1))
    tiny = ctx.enter_context(tc.tile_pool(name="tiny", bufs=4))
    att_pool = ctx.enter_context(tc.tile_pool(name="att", bufs=2))

    # PSUM: 8 banks total.  mm: 4 x [<=128, <=384] ; tr: 2 x [<=128, <=128] ;
    # sm: 2 x (skinny tiles)
    ps_mm = ctx.enter_context(tc.tile_pool(name="ps_mm", bufs=4, space="PSUM"))
    ps_tr = ctx.enter_context(tc.tile_pool(name="ps_tr", bufs=2, space="PSUM"))
    ps_sm = ctx.enter_context(tc.tile_pool(name="ps_sm", bufs=2, space="PSUM"))

    _cnt = [0]

    def _nm(pfx):
        _cnt[0] += 1
        return f"{pfx}{_cnt[0]}"

    def mm_tile(shape):
        return ps_mm.tile(shape, F32, tag="mm", name=_nm("mm"))

    def tr_tile(shape):
        return ps_tr.tile(shape, F32, tag="tr", name=_nm("tr"))

    def sm_tile(shape):
        return ps_sm.tile(shape, F32, tag="sm", name=_nm("sm"))

    def dmar(out_ap, in_ap):
        dma(_r(out_ap), _r(in_ap))

    # ---- input loads + projection first (PE starts ASAP) ----
    nkc = len(IN_CHUNKS)
    ones_row = const.tile([1, 128], F32)
    nc.vector.memset(ones_row[:], 1.0)
    h_pk = hpool.tile([NT, 2 * D], F32)   # packed [tok-chunk seg, D]
    with tc.tile_pool(name="xin", bufs=1) as xin:
        xT_s = xin.tile([128, nkc * N], F32)
        in_w_s = xin.tile([128, nkc * D], F32)
        in_b_s = xin.tile([1, D], F32)
        for kc, (k0, kw) in enumerate(IN_CHUNKS):
            dmar(xT_s[:kw, kc * N:(kc + 1) * N], inp["xT"][k0:k0 + kw, :])
            dmar(in_w_s[:kw, kc * D:(kc + 1) * D], inp["in_w"][k0:k0 + kw, :])
        dma(in_b_s[:], inp["in_b"][:])
        for seg in range(2):
            ps = mm_tile([NT, D])
            for kc, (k0, kw) in enumerate(IN_CHUNKS):
                nc.tensor.matmul(
                    ps[:],
                    _f(xT_s[:kw, kc * N + seg * NT: kc * N + seg * NT + NT]),
                    _f(in_w_s[:kw, kc * D:(kc + 1) * D]),
                    start=(kc == 0), stop=False,
                )
            nc.tensor.matmul(ps[:], ones_row[:, :NT], in_b_s[:],
                             start=False, stop=True)
            nc.vector.tensor_copy(h_pk[:, seg * D:(seg + 1) * D], ps[:])

    # ---- constants ----
    ident = const.tile([128, 128], F32)
    dma(ident[:], inp["ident"][:])
    ident_r = const.tile([128, 128], F32)
    dmar(ident_r[:], inp["ident"][:])
    ones_c96 = const.tile([96, 1], F32)
    nc.vector.memset(ones_c96[:], 1.0)
    eps_col = const.tile([128, 1], F32)
    nc.vector.memset(eps_col[:], 1e-6)
    epsD2_col = const.tile([128, 1], F32)
    nc.vector.memset(epsD2_col[:], 1e-6 * D * D)
    lnD_col = const.tile([128, 1], F32)
    nc.vector.memset(lnD_col[:], math.log(float(D)))



    def ln_modulate(src, dst):
        """dst = LN(src); packed [NT, 2*D] tiles (adaLN scale/shift folded
        into the per-batch weights on the host).  Critical path:
        reduce -> msq2 -> sub -> Ln -> Exp -> apply; centering runs parallel."""
        sums = tiny.tile([NT, 2], F32, tag="lnsum")
        sq = tiny.tile([NT, 2], F32, tag="lnsq")
        scr = work.tile([NT, 2 * D], F32, tag="lnscr")
        for seg in range(2):
            act(scr[:, seg * D:(seg + 1) * D], src[:, seg * D:(seg + 1) * D],
                AFT.Square, accum_out=sq[:, seg:seg + 1])
        nc.vector.reduce_sum(sums[:], src[:].rearrange("p (s d) -> p s d", s=2), axis=AX.X)
        # D^2*(var+eps) = D*sq - sums^2 + D^2*eps ; rstd via Ln/Exp with
        # the D factor and eps folded into the activation bias terms.
        msq2 = tiny.tile([NT, 2], F32, tag="lnmsq")
        TT(msq2[:], sums[:], sums[:], op=ALU.mult)
        var2 = tiny.tile([NT, 2], F32, tag="lnvar")
        TS(var2[:], sq[:], float(D), None, op0=ALU.mult)
        TT(var2[:], var2[:], msq2[:], op=ALU.subtract)
        lnv = tiny.tile([NT, 2], F32, tag="lnlnv")
        act(lnv[:], var2[:], AFT.Ln, bias=epsD2_col[:NT, :])
        rstd = tiny.tile([NT, 2], F32, tag="lnrstd")
        act(rstd[:], lnv[:], AFT.Exp, scale=-0.5, bias=lnD_col[:NT, :])
        # centering (parallel with the var chain)
        m = tiny.tile([NT, 2], F32, tag="lnm")
        TS(m[:], sums[:], 1.0 / D, None, op0=ALU.mult)
        ctr = work.tile([NT, 2 * D], F32, tag="lnctr")
        c3 = ctr[:].rearrange("p (s d) -> p s d", s=2)
        TT(c3, src[:].rearrange("p (s d) -> p s d", s=2),
           m[:, :, None].broadcast_to((NT, 2, D)), op=ALU.subtract)
        for seg in range(2):
            act(dst[:, seg * D:(seg + 1) * D], ctr[:, seg * D:(seg + 1) * D],
                AFT.Identity, scale=rstd[:, seg:seg + 1])

    def transpose4(src_pk, dst0, dst1):
        """src packed [NT, 2*D] -> two [128, N] tiles (feature-chunk major)."""
        for hc in range(2):
            for seg in range(2):
                tp = tr_tile([128, NT])
                nc.tensor.transpose(
                    tp[:], src_pk[:, seg * D + hc * 128: seg * D + (hc + 1) * 128],
                    ident[:NT, :NT])
                dst = dst0 if hc == 0 else dst1
                nc.vector.tensor_copy(dst[:, seg * NT:(seg + 1) * NT], tp[:])

    # ================= DiT backbone =================
    for l in range(LAYERS):
        p = lwl[l]
        qkv_b_s = lsmall.tile([128, 6], F32, tag="qkvb")
        dma(qkv_b_s[:], p["qkv_b"][:])
        qk_s = lqkv.tile([128, 2 * 2 * D], BF16, tag="qkw")
        for kc in range(2):
            dma(qk_s[:, kc * 2 * D:(kc + 1) * 2 * D], p["qk_bf"][kc * 128:(kc + 1) * 128, :])
        vw_s = lqkv.tile([128, 2 * D], BF16, tag="vw")
        for kc in range(2):
            dma(vw_s[:, kc * D:(kc + 1) * D], p["v_bf"][kc * 128:(kc + 1) * 128, :])
        v_b_s = lsmall.tile([1, D], F32, tag="vbs")
        dma(v_b_s[:], p["vb"][:])
        o_b_s = lsmall.tile([1, D], F32, tag="obs")
        dma(o_b_s[:], p["o_b"][:])
        o_w_s = lo.tile([128, 2 * D], F32, tag="ow")
        for kc in range(2):
            dmar(o_w_s[:, kc * D:(kc + 1) * D], p["o_w"][kc * 128:(kc + 1) * 128, :])
        m1_b_s = lsmall.tile([128, 8], F32, tag="m1b")
        dma(m1_b_s[:], p["m1_b"][:])
        m1_s = lm1.tile([128, 2 * 4 * D], BF16, tag="m1w")
        for kc in range(2):
            dma(m1_s[:, kc * 4 * D:(kc + 1) * 4 * D], p["m1_bf"][kc * 128:(kc + 1) * 128, :])
        m2_b_s = lsmall.tile([1, D], F32, tag="m2bs")
        dma(m2_b_s[:], p["m2_b"][:])
        m2_s = lm2.tile([128, 8 * D], F32, tag="m2w")
        for kc in range(8):
            dmar(m2_s[:, kc * D:(kc + 1) * D], p["m2_w"][kc * 128:(kc + 1) * 128, :])

        # --- attention ---
        a_pk = work.tile([NT, 2 * D], F32, tag="a_pk")
        ln_modulate(h_pk, a_pk)
        aT0 = work.tile([128, N], BF16, tag="aT0")
        aT1 = work.tile([128, N], BF16, tag="aT1")
        transpose4(a_pk, aT0, aT1)
        aTs = [aT0, aT1]

        # qkT chunks (q feats 0:256, k feats 256:512)
        qkT = []
        for mc in range(4):
            ps = mm_tile([128, N])
            for kc in range(2):
                nc.tensor.matmul(
                    ps[:], qk_s[:, kc * 2 * D + mc * 128: kc * 2 * D + (mc + 1) * 128],
                    aTs[kc][:], start=(kc == 0), stop=(kc == 1))
            sb = att_pool.tile([128, N], BF16, tag=f"qkT{mc}")
            if mc < 2:
                TS(sb[:], ps[:], qkv_b_s[:, mc:mc + 1], 1.0 / math.sqrt(HD),
                   op0=ALU.add, op1=ALU.mult)
            else:
                TS(sb[:], ps[:], qkv_b_s[:, mc:mc + 1], None, op0=ALU.add)
            qkT.append(sb)

        # v token-major [NT, D] x2 segments (+ bias)
        v_s = []
        for seg in range(2):
            ps = mm_tile([NT, D])
            for kc in range(2):
                nc.tensor.matmul(
                    ps[:], aTs[kc][:, seg * NT:(seg + 1) * NT],
                    vw_s[:, kc * D:(kc + 1) * D],
                    start=(kc == 0), stop=False)
            nc.tensor.matmul(ps[:], ones_row[:, :NT], v_b_s[:],
                             start=False, stop=True)
            vt = att_pool.tile([NT, D], F32, tag=f"v{seg}")
            nc.vector.tensor_copy(vt[:], ps[:])
            v_s.append(vt)

        # scores^T per head (no transposes); softmax normalisation deferred:
        # attTu = exp(scores^T) unnormalised, o scaled by 1/rowsum at copy-out.
        attTu = []
        rr_h = []
        for h in range(HEADS):
            qh = qkT[h // 2][(h % 2) * HD:(h % 2 + 1) * HD, :]
            kh = qkT[2 + h // 2][(h % 2) * HD:(h % 2 + 1) * HD, :]
            ps = mm_tile([NT, 2 * N])   # [ktok-chunk km on partitions, (km, qtok)]
            for km in range(2):
                nc.tensor.matmul(ps[:, km * N:(km + 1) * N],
                                 kh[:, km * NT:(km + 1) * NT], qh[:],
                                 start=True, stop=True)
            eT = att_pool.tile([NT, 2 * N], F32, tag=f"attTu{h}", name=f"attTu{h}")
            act(eT[:], ps[:], AFT.Exp)
            rr = tiny.tile([NT, 2], F32, tag=f"rr{h}", name=f"rr{h}")
            for qm in range(2):
                cs = sm_tile([NT, 1])
                for km in range(2):
                    nc.tensor.matmul(cs[:], eT[:, km * N + qm * NT: km * N + (qm + 1) * NT],
                                     ones_c96[:], start=(km == 0), stop=(km == 1))
                nc.vector.reciprocal(rr[:, qm:qm + 1], cs[:])
            attTu.append(eT)
            rr_h.append(rr)

        # o = softmax(att) @ v  (token-major): unnormalised matmul + scaled copy
        o_sb = []
        for qm in range(2):
            ps = mm_tile([NT, D])
            for h in range(HEADS):
                for km in range(2):
                    nc.tensor.matmul(
                        ps[:, h * HD:(h + 1) * HD],
                        attTu[h][:, km * N + qm * NT: km * N + (qm + 1) * NT],
                        v_s[km][:, h * HD:(h + 1) * HD],
                        start=(km == 0), stop=(km == 1))
            ot = work.tile([NT, D], F32, tag=f"o{qm}")
            for h in range(HEADS):
                TS(ot[:, h * HD:(h + 1) * HD], ps[:, h * HD:(h + 1) * HD],
                   rr_h[h][:, qm:qm + 1], None, op0=ALU.mult)
            o_sb.append(ot)
        oT0 = work.tile([128, N], F32, tag="oT0")
        oT1 = work.tile([128, N], F32, tag="oT1")
        for hc in range(2):
            for qm in range(2):
                tp = tr_tile([128, NT])
                nc.tensor.transpose(tp[:], o_sb[qm][:, hc * 128:(hc + 1) * 128], ident[:NT, :NT])
                dst = oT0 if hc == 0 else oT1
                nc.vector.tensor_copy(_r(dst[:, qm * NT:(qm + 1) * NT]), tp[:])
        oTs = [oT0, oT1]

        # h += g1 * (o @ o_w + o_b)
        for seg in range(2):
            ps = mm_tile([NT, D])
            for kc in range(2):
                nc.tensor.matmul(ps[:], _f(oTs[kc][:, seg * NT:(seg + 1) * NT]),
                                 _f(o_w_s[:, kc * D:(kc + 1) * D]),
                                 start=(kc == 0), stop=False)
            nc.tensor.matmul(ps[:], ones_row[:, :NT], o_b_s[:],
                             start=False, stop=True)
            TT(h_pk[:, seg * D:(seg + 1) * D], h_pk[:, seg * D:(seg + 1) * D], ps[:], op=ALU.add)

        # --- MLP ---
        m_pk = work.tile([NT, 2 * D], F32, tag="m_pk")
        ln_modulate(h_pk, m_pk)
        mT0 = work.tile([128, N], BF16, tag="mT0")
        mT1 = work.tile([128, N], BF16, tag="mT1")
        transpose4(m_pk, mT0, mT1)
        mTs = [mT0, mT1]
        m1g = []
        for mc in range(8):
            ps = mm_tile([128, N])
            for kc in range(2):
                nc.tensor.matmul(ps[:], m1_s[:, kc * 4 * D + mc * 128: kc * 4 * D + (mc + 1) * 128],
                                 mTs[kc][:], start=(kc == 0), stop=(kc == 1))
            g = att_pool.tile([128, N], F32, tag=f"m1g{mc}")
            act(_r(g[:]), ps[:], AFT.Gelu_apprx_tanh, bias=m1_b_s[:, mc:mc + 1])
            m1g.append(g)
        for seg in range(2):
            ps = mm_tile([NT, D])
            for kc in range(8):
                nc.tensor.matmul(ps[:], _f(m1g[kc][:, seg * NT:(seg + 1) * NT]),
                                 _f(m2_s[:, kc * D:(kc + 1) * D]),
                                 start=(kc == 0), stop=False)
            nc.tensor.matmul(ps[:], ones_row[:, :NT], m2_b_s[:],
                             start=False, stop=True)
            TT(h_pk[:, seg * D:(seg + 1) * D], h_pk[:, seg * D:(seg + 1) * D], ps[:], op=ALU.add)

    S_s = const.tile([96, 2 * MY_ROWS], F32)   # seg-major chunks of S
    dma(S_s[:, 0:MY_ROWS], inp["S"][0:NT, :])
    dma(S_s[:, MY_ROWS:], inp["S"][NT:N, :])

    # pairwise weights
    W0_s = wpool.tile([128, 2 * D], F32)
    for kc in range(2):
        dma(W0_s[:, kc * D:(kc + 1) * D], inp["W0"][kc * 128:(kc + 1) * 128, :])
    W12_s = wpool.tile([128, 4 * D], F32)
    for kc in range(4):
        dmar(W12_s[:, kc * D:(kc + 1) * D], inp["W12"][kc * 128:(kc + 1) * 128, :])
    p1_w_s = wpool.tile([128, 2 * MLP_HID], F32)
    for kc in range(2):
        dmar(p1_w_s[:, kc * MLP_HID:(kc + 1) * MLP_HID], inp["p1_w"][kc * 128:(kc + 1) * 128, :])
    p0_b_s = const.tile([128, 2], F32)
    dma(p0_b_s[:], inp["p0_b"][:])
    p1_b_s = const.tile([128, 2], F32)
    dma(p1_b_s[:], inp["p1_b"][:])
    po_w_s = const.tile([128, 2], F32)
    dmar(po_w_s[:], inp["po_w"][:])

    # ---- final LN -> hn ----
    hn_pk = hpool.tile([NT, 2 * D], F32)
    ln_modulate(h_pk, hn_pk)
    hnT0 = hpool.tile([128, N], F32)
    hnT1 = hpool.tile([128, N], F32)
    transpose4(hn_pk, hnT0, hnT1)
    hnTs = [hnT0, hnT1]

    # ---- row selection: hnT_mine = hn^T @ S (+ negated copy for Abs bias) ----
    hnT_mine, gTb_mine = [], []
    for hc in range(2):
        ps = tr_tile([128, MY_ROWS])
        for seg in range(2):
            nc.tensor.matmul(ps[:], hn_pk[:, seg * D + hc * 128: seg * D + (hc + 1) * 128],
                             S_s[:, seg * MY_ROWS:(seg + 1) * MY_ROWS],
                             start=(seg == 0), stop=(seg == 1))
        t = hpool.tile([128, MY_ROWS], F32, name=f"hnTm{hc}", tag=f"hnTm{hc}")
        nc.vector.tensor_copy(t[:], ps[:])
        hnT_mine.append(t)

    # ---- gT2 = (hn @ W0)^T duplicated along free dim; gTb_mine for gelu bias ----
    gT2 = []
    for hc in range(2):
        ps = mm_tile([128, N])
        for kc in range(2):
            nc.tensor.matmul(ps[:], W0_s[:, kc * D + hc * 128: kc * D + (hc + 1) * 128],
                             hnTs[kc][:], start=(kc == 0), stop=(kc == 1))
        g2t = hpool.tile([128, 2 * N], F32, name=f"gT2_{hc}", tag=f"gT2_{hc}")
        nc.vector.tensor_copy(_r(g2t[:, 0:N]), ps[:])
        nc.vector.tensor_copy(_r(g2t[:, N:2 * N]), ps[:])
        gT2.append(g2t)
    for hc in range(2):
        ps = tr_tile([128, MY_ROWS])
        for kc in range(2):
            nc.tensor.matmul(ps[:], W0_s[:, kc * D + hc * 128: kc * D + (hc + 1) * 128],
                             hnT_mine[kc][:], start=(kc == 0), stop=(kc == 1))
        t = hpool.tile([128, MY_ROWS], F32, name=f"gTbm{hc}", tag=f"gTbm{hc}")
        TS(_r(t[:]), ps[:], p0_b_s[:, hc:hc + 1], None, op0=ALU.add)
        gTb_mine.append(t)

    # ================= pairwise MLP =================
    pw_ft = ctx.enter_context(tc.tile_pool(name="pw_ft", bufs=2))
    pw_h = ctx.enter_context(tc.tile_pool(name="pw_h", bufs=2))
    strip_pool = ctx.enter_context(tc.tile_pool(name="strips", bufs=4))

    def _exchange(half):
        if skip_collective:
            # timing-sim stand-in: same bytes moved, no cross-core semantics
            dma(xch_out[half][0:MY_ROWS // 2, :], xch_in[half][:])
            dma(xch_out[half][MY_ROWS // 2:, :], xch_in[half][:])
        else:
            nc.gpsimd.collective_compute(
                "AllGather", ALU.bypass,
                replica_groups=[[0, 1], [2, 3], [4, 5], [6, 7]],
                ins=[xch_in[half][:]],
                outs=[xch_out[half][:]],
            )

    for blk in range(NBLK):
        i0 = blk * IB
        # features: ft[0..1] = |hn_j - hn_i| (hid chunks), ft[2..3] = hn_j * hn_i
        fts = []
        for hc in range(2):
            scr = pw_ft.tile([128, PWN], F32, tag="ftscr", name=_nm("ftscr"))
            for s in range(IB):
                TS(scr[:, s * N:(s + 1) * N], hnTs[hc][:],
                   hnT_mine[hc][:, i0 + s:i0 + s + 1], None, op0=ALU.subtract)
            ft = pw_ft.tile([128, PWN], F32, tag=f"ftA{hc}")
            # |x| as a size-1-axis max-reduce with apply_absolute_value
            nc.vector.tensor_reduce(
                _r(ft[:])[:, :, None], scr[:][:, :, None], axis=AX.X,
                op=ALU.max, apply_absolute_value=True)
            fts.append(ft)
        for hc in range(2):
            ft = pw_ft.tile([128, PWN], F32, tag=f"ftM{hc}")
            for s in range(IB):
                TS(_r(ft[:, s * N:(s + 1) * N]), hnTs[hc][:],
                   hnT_mine[hc][:, i0 + s:i0 + s + 1], None, op0=ALU.mult)
            fts.append(ft)

        # p0: psum[mc] = I @ gT2[mc] + sum_kc W12[kc,mc] @ ft[kc]
        h1 = []
        for mc in range(2):
            ps = mm_tile([128, PWN])
            nc.tensor.matmul(ps[:], _f(ident_r[:]), _f(gT2[mc][:]), start=True, stop=False)
            gtb_rep = gTb_mine[mc][:, i0:i0 + IB, None].broadcast_to((128, IB, N))
            nc.tensor.matmul(ps[:].rearrange("p (s j) -> p s j", s=IB),
                             _f(ident_r[:]), _f(gtb_rep), start=False, stop=False)
            for kc in range(4):
                nc.tensor.matmul(ps[:], _f(W12_s[:, kc * D + mc * 128: kc * D + (mc + 1) * 128]),
                                 _f(fts[kc][:]), start=False, stop=(kc == 3))
            g = pw_h.tile([128, PWN], F32, tag=f"h1_{mc}")
            act(_r(g[:]), ps[:], AFT.Gelu_apprx_tanh)
            h1.append(g)

        # p1 -> gelu -> h2
        h2 = []
        for mc in range(2):
            ps = mm_tile([128, PWN])
            for kc in range(2):
                nc.tensor.matmul(ps[:], _f(p1_w_s[:, kc * MLP_HID + mc * 128: kc * MLP_HID + (mc + 1) * 128]),
                                 _f(h1[kc][:]), start=(kc == 0), stop=(kc == 1))
            g = pw_h.tile([128, PWN], F32, tag=f"h2_{mc}")
            act(_r(g[:]), ps[:], AFT.Gelu_apprx_tanh, bias=p1_b_s[:, mc:mc + 1])
            h2.append(g)

        # po -> logits strip [1, PWN] -> DRAM exchange buffer rows
        ps = sm_tile([1, PWN])
        for kc in range(2):
            nc.tensor.matmul(ps[:], _f(po_w_s[:, kc:kc + 1]), _f(h2[kc][:]),
                             start=(kc == 0), stop=(kc == 1))
        strip = strip_pool.tile([1, PWN], F32, tag="postrip")
        nc.vector.tensor_copy(strip[:], ps[:])
        half, r0 = divmod(i0, MY_ROWS // 2)
        dma(xch_in[half][r0:r0 + IB, :].rearrange("r c -> (r c)")[None, :], strip[:])
        if blk == NBLK // 2 - 1:
            _exchange(0)

    # ---- second half exchange (first was issued mid-loop) ----
    _exchange(1)

    # ================= Sinkhorn (log domain, packed rows [NT, 2*N]) ==========
    sk = ctx.enter_context(tc.tile_pool(name="sink", bufs=1))
    skw = ctx.enter_context(tc.tile_pool(name="sinkw", bufs=2))

    A = sk.tile([NT, 2 * N], F32)
    HR = MY_ROWS // 2
    A3 = A[:].rearrange("p (s j) -> p s j", s=2)
    # gathered halves: out_a = global rows {0:48} u {96:144}, out_b = {48:96} u {144:192}
    for s in range(2):
        dma(A3[0:HR, s, :], xch_out[0][s * HR:(s + 1) * HR, :])
        dma(A3[HR:2 * HR, s, :], xch_out[1][s * HR:(s + 1) * HR, :])
    M0p_s = sk.tile([NT, 2 * N], F32)
    dma(M0p_s[:], inp["M0p"][:])
    M1p_s = sk.tile([NT, 2 * N], F32)
    dma(M1p_s[:], inp["M1p"][:])
    M1f_s = sk.tile([NT, 2 * N], F32)
    dma(M1f_s[:], inp["M1fix"][:])
    TT(A[:], A[:], M0p_s[:], op=ALU.mult)
    TT(A[:], A[:], M1p_s[:], op=ALU.add)

    for it in range(SINK_ITERS):
        # row step: E = exp(A) with fused per-segment rowsums (accum_out)
        E = skw.tile([NT, 2 * N], F32, tag="E")
        rs = skw.tile([NT, 2], F32, tag="rs")
        for s in range(2):
            act(E[:, s * N:(s + 1) * N], A[:, s * N:(s + 1) * N], AFT.Exp,
                accum_out=rs[:, s:s + 1])
        lr = skw.tile([NT, 2], F32, tag="lr")
        act(lr[:], rs[:], AFT.Ln)
        rcp = skw.tile([NT, 2], F32, tag="rcp")
        nc.vector.reciprocal(rcp[:], rs[:])

        # col sums of the row-normalised matrix, with the row scaling fused
        # into the matmul: cs = sum_s rcp[:,s]^T @ E[:,s]  ([1,192] psum)
        csp = sm_tile([1, N])
        for s in range(2):
            nc.tensor.matmul(csp[:], rcp[:, s:s + 1], E[:, s * N:(s + 1) * N],
                             start=(s == 0), stop=(s == 1))
        lrow = skw.tile([1, N], F32, tag="lrow")
        act(lrow[:], csp[:], AFT.Ln)

        # A -= (lr broadcast along cols) + (ln colsums broadcast along rows)
        bc = mm_tile([NT, 2 * N])
        nc.tensor.matmul(bc[:], ident[:NT, :NT],
                         lr[:, :, None].broadcast_to((NT, 2, N)),
                         start=True, stop=False)
        for s in range(2):
            nc.tensor.matmul(bc[:, s * N:(s + 1) * N], ones_row[:, :NT], lrow[:],
                             start=False, stop=(s == 1))
        for s in range(2):
            TT(A[:, s * N:(s + 1) * N], A[:, s * N:(s + 1) * N],
               bc[:, s * N:(s + 1) * N], op=ALU.subtract)

    # ---- outputs ----
    la_pk = sk.tile([NT, 2 * N], F32)
    TT(la_pk[:], A[:], M1f_s[:], op=ALU.add)
    dma(la_d[:].rearrange("(s p) j -> p s j", s=2), la_pk[:].rearrange("p (s j) -> p s j", s=2))

    E = skw.tile([NT, 2 * N], F32, tag="E")
    act(E[:], A[:], AFT.Exp)
    x0_pk = sk.tile([NT, 2 * N], F32)
    for s in range(2):
        for cb in range(2):
            tp = tr_tile([96, 96])
            nc.tensor.transpose(tp[:], E[:, cb * N + s * 96: cb * N + (s + 1) * 96], ident[:96, :96])
            TT(x0_pk[:, s * N + cb * 96: s * N + (cb + 1) * 96],
               E[:, s * N + cb * 96: s * N + (cb + 1) * 96], tp[:], op=ALU.add)
    TS(x0_pk[:], x0_pk[:], 0.5, None, op0=ALU.mult)
    dma(x0_d[:].rearrange("(s p) j -> p s j", s=2), x0_pk[:].rearrange("p (s j) -> p s j", s=2))


# --------------------------------------------------------------------------
# Host side
# --------------------------------------------------------------------------

def _silu(x):
    return x / (1.0 + np.exp(-x))


def _t_embed_np(t, dim):
    half = dim // 2
    freqs = np.exp(-np.log(10000.0) * np.arange(half, dtype=np.float32) / half)
    a = t.astype(np.float32)[:, None] * freqs[None, :]
    return np.concatenate([np.cos(a), np.sin(a)], axis=-1).astype(np.float32)


def host_inputs(x, timesteps, params, n_cores=8):
    """Build per-core in_maps. Core c handles batch c//2, row-half c%2."""
    x = np.asarray(x, np.float32)
    timesteps = np.asarray(timesteps)
    blocks = [{k: np.asarray(v, np.float32) for k, v in blk.items()} for blk in params["blocks"]]
    P = {k: np.asarray(v, np.float32) for k, v in params.items() if k != "blocks"}

    # conditioning chain (host): t-embed -> t1 -> t2; adaLN scale/shift/gate
    # folded into per-batch weights (exact reassociation: a@W = ln@(diag(u)W),
    # shift enters the bias, output gates scale weight columns).
    c = _t_embed_np(timesteps, D)
    c = _silu(c @ P["t1_w"] + P["t1_b"])
    c = c @ P["t2_w"] + P["t2_b"]
    sc = _silu(c)  # [B, D]
    import ml_dtypes
    folded = []  # [B][L] dict of per-batch layer tensors
    for b in range(B):
        per_l = []
        for blk in blocks:
            mv = sc[b] @ blk["mod_w"] + blk["mod_b"]
            s1, sc1, g1, s2, sc2, g2 = np.split(mv.astype(np.float32), 6)
            u1, u2 = 1.0 + sc1, 1.0 + sc2
            qkv_w = blk["qkv_w"] * u1[:, None]
            qkv_b = blk["qkv_b"] + s1 @ blk["qkv_w"]
            m1_w = blk["m1_w"] * u2[:, None]
            m1_b = blk["m1_b"] + s2 @ blk["m1_w"]
            o_w = blk["o_w"] * g1[None, :]
            o_b = blk["o_b"] * g1
            m2_w = blk["m2_w"] * g2[None, :]
            m2_b = blk["m2_b"] * g2
            per_l.append(dict(
                qk_bf=np.ascontiguousarray(qkv_w[:, :2 * D].astype(ml_dtypes.bfloat16)),
                v_bf=np.ascontiguousarray(qkv_w[:, 2 * D:].astype(ml_dtypes.bfloat16)),
                qkv_b=np.ascontiguousarray(qkv_b.reshape(6, 128).T, np.float32),
                vb=np.ascontiguousarray(qkv_b[2 * D:][None, :], np.float32),
                o_w=np.ascontiguousarray(o_w, np.float32),
                o_b=np.ascontiguousarray(o_b[None, :], np.float32),
                m1_bf=np.ascontiguousarray(m1_w.astype(ml_dtypes.bfloat16)),
                m1_b=np.ascontiguousarray(m1_b.reshape(8, 128).T, np.float32),
                m2_w=np.ascontiguousarray(m2_w, np.float32),
                m2_b=np.ascontiguousarray(m2_b[None, :], np.float32),
            ))
        folded.append(per_l)

    eye = np.eye(N, dtype=np.float32)
    po_b = float(P["po_b"].reshape(-1)[0])

    def pack_rows(mat):  # [N, N] -> [NT, 2N]  (row 96s+p at [p, s*N + j])
        return np.ascontiguousarray(
            mat.reshape(2, NT, N).transpose(1, 0, 2).reshape(NT, 2 * N), np.float32)

    M0 = (1.0 - eye) / TEMP
    M1 = po_b / TEMP * (1.0 - eye) + DIAG_SMALL * eye
    M1f = (-1e9 / TEMP - DIAG_SMALL) * eye

    def col_chunks(v, nchunk):  # [K] -> [128, nchunk] chunk-major
        return np.ascontiguousarray(v.reshape(nchunk, 128).T, np.float32)

    p0_w = P["p0_w"]  # [3D, MLP_HID]
    shared = {
        "in_w": np.ascontiguousarray(P["in_w"]),
        "in_b": np.ascontiguousarray(P["in_b"][None, :]),
        "W0": np.ascontiguousarray(p0_w[:D]),
        "W12": np.ascontiguousarray(p0_w[D:]),
        "p0_b": col_chunks(P["p0_b"], 2),
        "p1_w": np.ascontiguousarray(P["p1_w"]),
        "p1_b": col_chunks(P["p1_b"], 2),
        "po_w": col_chunks(P["po_w"].reshape(-1), 2),
        "ident": np.eye(128, dtype=np.float32),
        "M0p": pack_rows(M0), "M1p": pack_rows(M1), "M1fix": pack_rows(M1f),
    }

    eyeN = np.eye(N, dtype=np.float32)
    S_half = [np.ascontiguousarray(eyeN[:, :MY_ROWS]),
              np.ascontiguousarray(eyeN[:, MY_ROWS:])]

    in_maps = []
    for core in range(n_cores):
        b, half = core // 2, core % 2
        m = dict(shared)
        m["xT"] = np.ascontiguousarray(x[b].T)
        m["S"] = S_half[half]
        for l in range(LAYERS):
            for k, v in folded[b][l].items():
                m[f"L{l}_{k}"] = v
        in_maps.append(m)
    return in_maps


_CACHED = {}


def _get_program(n_cores=8):
    if n_cores not in _CACHED:
        _CACHED[n_cores] = build_program(n_cores=n_cores)
    return _CACHED[n_cores]


def run_hw(x, timesteps, params, trace=False, **kwargs):
    n_cores = 8
    nc = _get_program(n_cores)
    in_maps = host_inputs(x, timesteps, params, n_cores=n_cores)
    res = run_bass_kernel_spmd(nc, in_maps, core_ids=list(range(n_cores)),
                               trace=trace, **kwargs)
    la = np.stack([res.results[2 * b]["la_out"] for b in range(B)])
    x0 = np.stack([res.results[2 * b]["x0_out"] for b in range(B)])
    return (la.astype(np.float32), x0.astype(np.float32)), res


def kernel(x, timesteps, params):
    out, _ = run_hw(x, timesteps, params, trace=False)
    return out
